# revision 6
# baseline (speedup 1.0000x reference)
"""Trainium2 Bass kernel for nn_ByteEmbedding (segment_reduce).

Computation (per batch row, one row per NeuronCore, 8 cores):
  byte_emb = emb_weight[x] * sqrt(128)            # gather  [8192, 128]
  grouped  = segment_mean(byte_emb, byte_groups)  # ragged  [2048, 128]
  out      = grouped @ out_proj_w.T               # proj    [2048, 1024]

Device strategy:
  - dma_gather pulls emb rows for all 8192 positions into SBUF with
    positions on partitions: bemb[p, c, :] = emb[x[128c+p], :].
  - byte_groups is sorted, so each 128-position block touches only a
    couple of aligned 128-token regions. For each (block, region) pair a
    one-hot matrix oh[p, t] = (seg[p] == 128r + t) is built on DVE/ACT
    (iota + tensor_scalar is_equal), then PE accumulates
      sums^T[dim, region] += bemb_block.T @ oh      (PSUM accumulate)
    Counts come from the same one-hots via ones.T @ oh matmuls.
  - Mean division: counts row -> max(.,1) -> partition_broadcast ->
    reciprocal -> fused multiply with the PSUM->SBUF copy of sums^T.
  - sqrt(128) is folded into the transposed projection weights, which are
    transposed once on the PE.

The (block -> regions) structure is data dependent; kernel() reads the
integer index tensors on the host and specializes the program to the
union of the 8 rows' structures, so one SPMD program serves all cores.
"""

import os
import sys
from contextlib import ExitStack

import numpy as np

for _p in ("/opt/trn_rl_repo",):
    if _p not in sys.path and os.path.isdir(_p):
        sys.path.append(_p)

import concourse.bacc as bacc
import concourse.bass as bass
import concourse.mybir as mybir
import concourse.tile as tile
from concourse.bass_utils import run_bass_kernel_spmd
from concourse.masks import make_identity

B = 8
S = 8192          # bytes per row
V = 384           # vocab
D = 128           # byte dim
E = 1024          # out dim
T = 2048          # tokens
P = 128
NBLK = S // P     # 64 position blocks
NREG = T // P     # 16 token regions
NGRP = 4          # region groups (4 regions = 512 tokens each)
SCALE = float(D) ** 0.5
dt = mybir.dt

GATHER_CHUNKS = 4
IDX_PER_CHUNK = S // GATHER_CHUNKS  # 2048


def _structure(byte_groups: np.ndarray):
    """Union over rows of (block -> token regions) incidence."""
    bg = byte_groups.reshape(B, NBLK, P)
    lo = bg.min(axis=2) // P   # [B, NBLK]
    hi = bg.max(axis=2) // P
    block_regs = []
    for b in range(NBLK):
        r0 = int(lo[:, b].min())
        r1 = int(hi[:, b].max())
        block_regs.append(list(range(r0, r1 + 1)))
    reg_blocks = {r: [] for r in range(NREG)}
    for b, regs in enumerate(block_regs):
        for r in regs:
            reg_blocks[r].append(b)
    return reg_blocks


def _build(reg_blocks) -> bacc.Bacc:
    nc = bacc.Bacc(
        "TRN2",
        target_bir_lowering=False,
        debug=False,
        enable_asserts=True,
        num_devices=B,
    )

    x_tr = nc.dram_tensor("x_tr", [P, NBLK], dt.int32, kind="ExternalInput")
    bg_tr = nc.dram_tensor("bg_tr", [P, NBLK], dt.int32, kind="ExternalInput")
    emb_weight = nc.dram_tensor("emb_weight", [V, D], dt.float32, kind="ExternalInput")
    out_proj_w = nc.dram_tensor("out_proj_w", [E, D], dt.float32, kind="ExternalInput")
    out = nc.dram_tensor("out", [T, E], dt.float32, kind="ExternalOutput")

    with tile.TileContext(nc) as tc, ExitStack() as ctx:
        sb = ctx.enter_context(tc.tile_pool(name="sb", bufs=1))
        oh_pool = ctx.enter_context(tc.tile_pool(name="oh", bufs=40))
        grp_pool = ctx.enter_context(tc.tile_pool(name="grp", bufs=2))
        cnt_pool = ctx.enter_context(tc.tile_pool(name="cnt", bufs=2))
        outsb_pool = ctx.enter_context(tc.tile_pool(name="outsb", bufs=4))
        ps_sums = ctx.enter_context(tc.tile_pool(name="ps_sums", bufs=2, space="PSUM"))
        ps_cnt = ctx.enter_context(tc.tile_pool(name="ps_cnt", bufs=2, space="PSUM"))
        ps_out = ctx.enter_context(tc.tile_pool(name="ps_out", bufs=3, space="PSUM"))

        # ---- inputs to SBUF ----
        xts = sb.tile([P, NBLK], dt.int32, name="xts")
        nc.sync.dma_start(out=xts[:], in_=x_tr.ap())

        seg_i = sb.tile([P, NBLK], dt.int32, name="seg_i")
        nc.sync.dma_start(out=seg_i[:], in_=bg_tr.ap())
        seg_f = sb.tile([P, NBLK], dt.float32, name="seg_f")
        nc.any.tensor_copy(out=seg_f[:], in_=seg_i[:])

        # ---- embedding gather: bemb[ci][p, c, :] = emb[x[2048*ci + 128*(16*ci+c) ... ]]
        # One stock indirect DMA per 128-position block: bemb[ci][p, c, :] =
        # emb[x_tr[p, 16*ci + c]] = emb[x[128*(16*ci+c) + p]].
        bemb = []
        for ci in range(GATHER_CHUNKS):
            bt = sb.tile([P, NBLK // GATHER_CHUNKS, D], dt.float32, name=f"bemb{ci}")
            for c in range(NBLK // GATHER_CHUNKS):
                b = ci * (NBLK // GATHER_CHUNKS) + c
                nc.gpsimd.indirect_dma_start(
                    out=bt[:, c, :], out_offset=None, in_=emb_weight.ap(),
                    in_offset=bass.IndirectOffsetOnAxis(ap=xts[:, b:b + 1], axis=0),
                )
            bemb.append(bt)

        def bemb_block(b):
            return bemb[b // 16][:, b % 16, :]

        # ---- constants ----
        iota_f = sb.tile([P, T], dt.float32, name="iota_f")
        nc.gpsimd.iota(
            iota_f[:], pattern=[[1, T]], base=0, channel_multiplier=0,
            allow_small_or_imprecise_dtypes=True,
        )
        identity = sb.tile([P, P], dt.float32, name="identity")
        make_identity(nc, identity[:])
        ones_t = sb.tile([P, 1], dt.float32, name="ones_t")
        nc.vector.memset(ones_t[:], 1.0)

        # ---- WT = out_proj_w.T * SCALE, via PE transpose ----
        wt_sb = sb.tile([P, E], dt.float32, name="wt_sb")
        for j in range(E // P):
            wtile = sb.tile([P, P], dt.float32, name="wtile", tag="wtile", bufs=2)
            nc.sync.dma_start(out=wtile[:], in_=out_proj_w.ap()[j * P:(j + 1) * P, :])
            pst = ps_out.tile([P, P], dt.float32, name="pst", tag="pso")
            nc.tensor.transpose(out=pst[:], in_=wtile[:], identity=identity[:])
            nc.any.tensor_scalar(
                out=wt_sb[:, j * P:(j + 1) * P], in0=pst[:],
                scalar1=SCALE, scalar2=None, op0=mybir.AluOpType.mult,
            )

        # ---- main loop over region groups (512 tokens each) ----
        for g in range(NGRP):
            regs = list(range(4 * g, 4 * g + 4))
            psum_s = ps_sums.tile([P, 512], dt.float32, name="psum_s")
            psum_c = ps_cnt.tile([1, 512], dt.float32, name="psum_c")

            ohs = {}
            for r in regs:
                col = (r % 4) * P
                blocks = reg_blocks[r]
                if not blocks:
                    nc.vector.memset(psum_s[:, col:col + P], 0.0)
                    nc.vector.memset(psum_c[:, col:col + P], 0.0)
                    continue
                for k, b in enumerate(blocks):
                    oh = oh_pool.tile([P, P], dt.float32, name="oh", tag="oh")
                    nc.any.tensor_scalar(
                        out=oh[:], in0=iota_f[:, r * P:(r + 1) * P],
                        scalar1=seg_f[:, b:b + 1], scalar2=None,
                        op0=mybir.AluOpType.is_equal,
                    )
                    ohs[(b, r)] = oh
                    nc.tensor.matmul(
                        out=psum_s[:, col:col + P],
                        lhsT=bemb_block(b), rhs=oh[:],
                        start=(k == 0), stop=(k == len(blocks) - 1),
                    )
            # counts via ones.T @ oh (single stationary vector, batched)
            for r in regs:
                col = (r % 4) * P
                blocks = reg_blocks[r]
                for k, b in enumerate(blocks):
                    nc.tensor.matmul(
                        out=psum_c[0:1, col:col + P],
                        lhsT=ones_t[:], rhs=ohs[(b, r)][:],
                        start=(k == 0), stop=(k == len(blocks) - 1),
                    )

            # counts -> reciprocal, broadcast down partitions
            cnt_row = cnt_pool.tile([1, 512], dt.float32, name="cnt_row", tag="cnt_row")
            nc.vector.tensor_scalar(
                out=cnt_row[:], in0=psum_c[:],
                scalar1=1.0, scalar2=None, op0=mybir.AluOpType.max,
            )
            cnt_bc = cnt_pool.tile([P, 512], dt.float32, name="cnt_bc", tag="cnt_bc")
            nc.gpsimd.partition_broadcast(cnt_bc[:], cnt_row[:])
            recip_bc = cnt_pool.tile([P, 512], dt.float32, name="recip_bc", tag="recip_bc")
            nc.vector.reciprocal(out=recip_bc[:], in_=cnt_bc[:])

            # grouped^T = sums^T * recip (fused into the PSUM->SBUF copy)
            grp_sb = grp_pool.tile([P, 512], dt.float32, name="grp_sb")
            nc.any.tensor_tensor(
                out=grp_sb[:], in0=psum_s[:], in1=recip_bc[:],
                op=mybir.AluOpType.mult,
            )

            # projection + store
            for r in regs:
                col = (r % 4) * P
                for h in range(2):
                    pso = ps_out.tile([P, 512], dt.float32, name="pso", tag="pso")
                    nc.tensor.matmul(
                        out=pso[:],
                        lhsT=grp_sb[:, col:col + P],
                        rhs=wt_sb[:, h * 512:(h + 1) * 512],
                        start=True, stop=True,
                    )
                    osb = outsb_pool.tile([P, 512], dt.float32, name="osb")
                    nc.any.tensor_copy(out=osb[:], in_=pso[:])
                    nc.sync.dma_start(
                        out=out.ap()[r * P:(r + 1) * P, h * 512:(h + 1) * 512],
                        in_=osb[:],
                    )

    nc.compile()
    return nc


def _prep_inputs(x, byte_groups, emb_weight, out_proj_w):
    """Host-side integer index plumbing (no float math)."""
    in_maps = []
    for k in range(B):
        xtr = x[k].reshape(NBLK, P).T.astype(np.int32)            # x_tr[p, c] = x[128 c + p]
        bgt = byte_groups[k].reshape(NBLK, P).T.astype(np.int32)  # bg_tr[p, c] = bg[128 c + p]
        in_maps.append({
            "x_tr": np.ascontiguousarray(xtr),
            "bg_tr": np.ascontiguousarray(bgt),
            "emb_weight": np.asarray(emb_weight, dtype=np.float32),
            "out_proj_w": np.asarray(out_proj_w, dtype=np.float32),
        })
    return in_maps


def _run(x, byte_groups, emb_weight, out_proj_w, trace=False, **kw):
    x = np.asarray(x)
    byte_groups = np.asarray(byte_groups)
    # The gather maps element j = 128c + p of the wrapped index stream to
    # bemb[p, c]; with idxs[g, s] = x[16s+g] the stream is x in order, so
    # position i = 128c + p  ->  partition p, block c.  bg_tr matches.
    reg_blocks = _structure(byte_groups)
    nc = _build(reg_blocks)
    in_maps = _prep_inputs(x, byte_groups, emb_weight, out_proj_w)
    res = run_bass_kernel_spmd(nc, in_maps, core_ids=list(range(B)), trace=trace, **kw)
    outs = np.stack([res.results[k]["out"] for k in range(B)], axis=0)
    return outs, res


def kernel(x, byte_groups, emb_weight, out_proj_w):
    outs, _ = _run(x, byte_groups, emb_weight, out_proj_w, trace=False)
    return outs
